# revision 39
# baseline (speedup 1.0000x reference)
"""BinaryLinear kernel for Trainium2 (8 NeuronCores, SPMD).

Computes  out = sign(x) @ sign(W)^T * alpha  for
x: [8192, 2048] f32, W: [2048, 2048] f32, alpha: [1] f32.

Strategy: data-parallel over the token dim (8 shards of 1024 tokens);
W replicated. Host side packs inputs into flat per-chunk streams so
every DMA is a single fully-contiguous transfer with 4-8 KB runs per
SBUF partition, in exact consumption order. On device: sign() both
operands into resident fp8(E4M3) SBUF buffers (+-1 exact; accumulation
of <=2048 +-1 terms is exact in fp32 PSUM), then DoubleRow fp8 matmuls
(2 k-tiles per MM), scale by alpha on PSUM drain (DVE/ACT
alternating), write out per m-pair (contiguous staging layout, host
re-merges).

Rings: the sync (HWDGE) ring carries all W chunks in strict
consumption order (n0 small chunks interleaved with x by k-progress,
then n1, n2, n3 quads); the scalar (HWDGE) ring carries alpha, all x
chunks, then the output writes (gated by drains).
"""

import numpy as np

import concourse.bass as bass
import concourse.tile as tile
from concourse import bacc, mybir
from concourse.bass_utils import run_bass_kernel_spmd

N_CORES = 8
NTOK = 8192
INF = 2048
OUTF = 2048
TPC = NTOK // N_CORES  # tokens per core (1024)
P = 128
KT = INF // P  # 16 contraction tiles
MT = TPC // P  # 8 token tiles per core
NTS = 512  # out_features per matmul (one PSUM bank)
NT = OUTF // NTS  # 4

F32 = mybir.dt.float32
FP8 = mybir.dt.float8e4  # E4M3; +-1.0 is exact
SIGN_DT = FP8
K_STEP = 2  # contraction tiles per matmul (2 = fp8 DoubleRow)

# W chunk schedule per n-slice: n0 in small chunks (fine-grained pacing
# while x streams, tiny first chunks to fill the pipeline), n1..n3 in
# k-quads (1 MiB chunks, 8 KB/partition runs).
W_CHUNKS = {0: [1, 1, 2, 2, 2, 4, 4], 1: [4] * 4, 2: [4] * 4, 3: [4] * 4}
X_CHUNKS = [1, 1, 2, 2, 2, 2, 2, 2, 2]

_compiled = None
LAST_RESULT = None  # BassKernelResults of the most recent run (for profiling)


def _build():
    nc = bacc.Bacc(
        "TRN2",
        target_bir_lowering=False,
        debug=False,
        num_devices=N_CORES,
    )
    xt = nc.dram_tensor("xt", [KT * P * TPC], F32, kind="ExternalInput").ap()
    wt = nc.dram_tensor("wt", [NT * KT * P * NTS], F32, kind="ExternalInput").ap()
    al = nc.dram_tensor("alpha", [P, 1], F32, kind="ExternalInput").ap()
    out = nc.dram_tensor(
        "out", [NT, MT // 2, P, 2 * NTS], F32, kind="ExternalOutput"
    ).ap()

    with tile.TileContext(nc) as tc:
        with (
            tc.tile_pool(name="res", bufs=1) as res,
            tc.tile_pool(name="wload", bufs=4) as wload,
            tc.tile_pool(name="xload", bufs=3) as xload,
            tc.tile_pool(name="psum", bufs=8, space="PSUM") as ppool,
            tc.tile_pool(name="outp", bufs=2) as outp,
        ):
            # Resident sign() buffers (fp8)
            bw = res.tile([P, KT, OUTF], SIGN_DT)  # 32 KB/partition
            bx = res.tile([P, KT, TPC], SIGN_DT)  # 16 KB/partition
            alpha_t = res.tile([P, 1], F32)

            perf_mode = mybir.MatmulPerfMode.DoubleRow if K_STEP == 2 else None

            def mm(ps_ap, m, n, k):
                nc.tensor.matmul(
                    ps_ap,
                    bx[:, k : k + K_STEP, m * P : (m + 1) * P],
                    bw[:, k : k + K_STEP, n * NTS : (n + 1) * NTS],
                    start=(k == 0),
                    stop=(k + K_STEP >= KT),
                    perf_mode=perf_mode,
                )

            w_off = [0]

            def load_sign_w_chunk(n, k0, sz, engine):
                wf = wload.tile([P, sz, NTS], F32, name="wf", tag="wf")
                src = wt[w_off[0] : w_off[0] + P * sz * NTS].rearrange(
                    "(p f) -> p f", p=P
                )
                engine.dma_start(wf[:].rearrange("p a b -> p (a b)"), src)
                w_off[0] += P * sz * NTS
                for j in range(sz):
                    nc.scalar.sign(bw[:, k0 + j, n * NTS : (n + 1) * NTS], wf[:, j, :])

            x_off = [0]

            def load_sign_x_chunk(k0, sz, engine):
                xf = xload.tile([P, sz, TPC], F32, name="xf", tag="xf")
                src = xt[x_off[0] : x_off[0] + P * sz * TPC].rearrange(
                    "(p f) -> p f", p=P
                )
                engine.dma_start(xf[:].rearrange("p a b -> p (a b)"), src)
                x_off[0] += P * sz * TPC
                for j in range(sz):
                    nc.vector.tensor_scalar(
                        bx[:, k0 + j, :], xf[:, j, :], 0.0, None,
                        op0=mybir.AluOpType.is_gt,
                    )
                    nc.vector.tensor_scalar(
                        bx[:, k0 + j, :], bx[:, k0 + j, :], 2.0, -1.0,
                        op0=mybir.AluOpType.mult, op1=mybir.AluOpType.add,
                    )

            # ---- load + sign phase (issue order == consumption order) ----
            # gpsimd ring: x chunks. sync ring: all W chunks, n0 first
            # (interleaved with x by k-progress), then n1, n2, n3.
            nc.gpsimd.dma_start(alpha_t[:], al)

            def next_w_ring():
                return nc.sync

            xi = wi = xk = wk = 0
            while xi < len(X_CHUNKS) or wi < len(W_CHUNKS[0]):
                if xi < len(X_CHUNKS) and (wi >= len(W_CHUNKS[0]) or xk <= wk):
                    load_sign_x_chunk(xk, X_CHUNKS[xi], nc.scalar)
                    xk += X_CHUNKS[xi]
                    xi += 1
                else:
                    load_sign_w_chunk(0, wk, W_CHUNKS[0][wi], next_w_ring())
                    wk += W_CHUNKS[0][wi]
                    wi += 1
            for n in (1, 2, 3):
                k0 = 0
                for sz in W_CHUNKS[n]:
                    load_sign_w_chunk(n, k0, sz, next_w_ring())
                    k0 += sz

            def drain(dst, ps, idx, last_pass):
                # DVE drains mid-kernel (ACT is busy signing); alternate
                # DVE/ACT in the last pass so the tail drains in parallel.
                if not last_pass or idx % 2 == 0:
                    nc.vector.tensor_scalar_mul(dst, ps, alpha_t[:])
                else:
                    nc.scalar.activation(
                        dst, ps, mybir.ActivationFunctionType.Copy,
                        scale=alpha_t[:],
                    )

            # ---- matmul phase ----
            for n in range(NT):
                obuf = outp.tile([P, MT, NTS], F32)
                if n < 2:
                    # streaming passes: k-middle / m-inner
                    pss = [
                        ppool.tile([P, NTS], F32, name="ps", tag="ps")
                        for _ in range(MT)
                    ]
                    for k in range(0, KT, K_STEP):
                        for m in range(MT):
                            mm(pss[m][:], m, n, k)
                    for m in range(MT):
                        drain(obuf[:, m, :], pss[m][:], m, n == NT - 1)
                        if m % 2 == 1:
                            nc.scalar.dma_start(
                                out[n, m // 2],
                                obuf[:, m - 1 : m + 1, :].rearrange(
                                    "p a b -> p (a b)"
                                ),
                            )
                else:
                    # resident passes: m-outer / k-inner
                    for m in range(MT):
                        ps = ppool.tile([P, NTS], F32, name="ps", tag="ps")
                        for k in range(0, KT, K_STEP):
                            mm(ps[:], m, n, k)
                        drain(obuf[:, m, :], ps[:], m, n == NT - 1)
                        if m % 2 == 1:
                            nc.scalar.dma_start(
                                out[n, m // 2],
                                obuf[:, m - 1 : m + 1, :].rearrange(
                                    "p a b -> p (a b)"
                                ),
                            )

    nc.compile()
    return nc


def _pack_w(weight):
    # WT4[k, p, n, c] = W^T[(k*128+p), n*512+c]
    wt4 = weight.T.reshape(KT, P, NT, NTS)
    parts = []
    for n in range(NT):
        k0 = 0
        for sz in W_CHUNKS[n]:
            parts.append(
                wt4[k0 : k0 + sz, :, n, :].transpose(1, 0, 2).ravel()
            )
            k0 += sz
    return np.ascontiguousarray(np.concatenate(parts))


def _pack_x_shard(xs):
    # xs: [TPC, INF] -> xT4[k, p, t]
    xt4 = xs.T.reshape(KT, P, TPC)
    parts = []
    k0 = 0
    for sz in X_CHUNKS:
        parts.append(xt4[k0 : k0 + sz].transpose(1, 0, 2).ravel())
        k0 += sz
    return np.ascontiguousarray(np.concatenate(parts))


def kernel(x, weight, alpha):
    global _compiled, LAST_RESULT
    if _compiled is None:
        _compiled = _build()
    nc = _compiled

    x = np.asarray(x, dtype=np.float32)
    weight = np.asarray(weight, dtype=np.float32)
    alpha = np.asarray(alpha, dtype=np.float32)

    wt = _pack_w(weight)
    alv = np.full((P, 1), alpha.reshape(-1)[0], dtype=np.float32)
    in_maps = []
    for c in range(N_CORES):
        xs = _pack_x_shard(x[c * TPC : (c + 1) * TPC, :])
        in_maps.append({"xt": xs, "wt": wt, "alpha": alv})

    LAST_RESULT = run_bass_kernel_spmd(nc, in_maps, list(range(N_CORES)))
    outs = []
    for c in range(N_CORES):
        o = LAST_RESULT.results[c]["out"]  # [NT, MT//2, P, 2*NTS]
        o = o.reshape(NT, MT // 2, P, 2, NTS)
        # -> [MT//2, 2, P, NT, NTS] -> [TPC, OUTF]
        outs.append(o.transpose(1, 3, 2, 0, 4).reshape(TPC, OUTF))
    return np.concatenate(outs, axis=0)


# revision 40
# speedup vs baseline: 1.0042x; 1.0042x over previous
"""BinaryLinear kernel for Trainium2 (8 NeuronCores, SPMD).

Computes  out = sign(x) @ sign(W)^T * alpha  for
x: [8192, 2048] f32, W: [2048, 2048] f32, alpha: [1] f32.

Strategy: data-parallel over the token dim (8 shards of 1024 tokens);
W replicated. Host side packs inputs into flat per-chunk streams so
every DMA is a single fully-contiguous transfer with 4-8 KB runs per
SBUF partition, in exact consumption order. On device: sign() both
operands into resident fp8(E4M3) SBUF buffers (+-1 exact; accumulation
of <=2048 +-1 terms is exact in fp32 PSUM), then DoubleRow fp8 matmuls
(2 k-tiles per MM), scale by alpha on PSUM drain (DVE/ACT
alternating), write out per m-pair (contiguous staging layout, host
re-merges).

Rings: the sync (HWDGE) ring carries all W chunks in strict
consumption order (n0 small chunks interleaved with x by k-progress,
then n1, n2, n3 quads); the scalar (HWDGE) ring carries alpha, all x
chunks, then the output writes (gated by drains).
"""

import numpy as np

import concourse.bass as bass
import concourse.tile as tile
from concourse import bacc, mybir
from concourse.bass_utils import run_bass_kernel_spmd

N_CORES = 8
NTOK = 8192
INF = 2048
OUTF = 2048
TPC = NTOK // N_CORES  # tokens per core (1024)
P = 128
KT = INF // P  # 16 contraction tiles
MT = TPC // P  # 8 token tiles per core
NTS = 512  # out_features per matmul (one PSUM bank)
NT = OUTF // NTS  # 4

F32 = mybir.dt.float32
FP8 = mybir.dt.float8e4  # E4M3; +-1.0 is exact
SIGN_DT = FP8
K_STEP = 2  # contraction tiles per matmul (2 = fp8 DoubleRow)

# W chunk schedule per n-slice: n0 in small chunks (fine-grained pacing
# while x streams, tiny first chunks to fill the pipeline), n1..n3 in
# k-quads (1 MiB chunks, 8 KB/partition runs).
W_CHUNKS = {0: [1, 1, 2, 2, 2, 4, 4], 1: [4] * 4, 2: [4] * 4, 3: [4] * 4}
X_CHUNKS = [1, 1, 2, 2, 2, 2, 2, 2, 2]

_compiled = None
LAST_RESULT = None  # BassKernelResults of the most recent run (for profiling)


def _build():
    nc = bacc.Bacc(
        "TRN2",
        target_bir_lowering=False,
        debug=False,
        num_devices=N_CORES,
    )
    xt = nc.dram_tensor("xt", [KT * P * TPC], F32, kind="ExternalInput").ap()
    wt = nc.dram_tensor("wt", [NT * KT * P * NTS], F32, kind="ExternalInput").ap()
    al = nc.dram_tensor("alpha", [P, 1], F32, kind="ExternalInput").ap()
    out = nc.dram_tensor(
        "out", [NT, MT // 2, P, 2 * NTS], F32, kind="ExternalOutput"
    ).ap()

    with tile.TileContext(nc) as tc:
        with (
            tc.tile_pool(name="res", bufs=1) as res,
            tc.tile_pool(name="wload", bufs=4) as wload,
            tc.tile_pool(name="xload", bufs=3) as xload,
            tc.tile_pool(name="psum", bufs=8, space="PSUM") as ppool,
            tc.tile_pool(name="outp", bufs=2) as outp,
        ):
            # Resident sign() buffers (fp8)
            bw = res.tile([P, KT, OUTF], SIGN_DT)  # 32 KB/partition
            bx = res.tile([P, KT, TPC], SIGN_DT)  # 16 KB/partition
            alpha_t = res.tile([P, 1], F32)

            perf_mode = mybir.MatmulPerfMode.DoubleRow if K_STEP == 2 else None

            def mm(ps_ap, m, n, k):
                nc.tensor.matmul(
                    ps_ap,
                    bx[:, k : k + K_STEP, m * P : (m + 1) * P],
                    bw[:, k : k + K_STEP, n * NTS : (n + 1) * NTS],
                    start=(k == 0),
                    stop=(k + K_STEP >= KT),
                    perf_mode=perf_mode,
                )

            w_off = [0]

            def load_sign_w_chunk(n, k0, sz, engine):
                wf = wload.tile([P, sz, NTS], F32, name="wf", tag="wf")
                src = wt[w_off[0] : w_off[0] + P * sz * NTS].rearrange(
                    "(p f) -> p f", p=P
                )
                engine.dma_start(wf[:].rearrange("p a b -> p (a b)"), src)
                w_off[0] += P * sz * NTS
                for j in range(sz):
                    nc.scalar.sign(bw[:, k0 + j, n * NTS : (n + 1) * NTS], wf[:, j, :])

            x_off = [0]

            def load_sign_x_chunk(k0, sz, engine):
                xf = xload.tile([P, sz, TPC], F32, name="xf", tag="xf")
                src = xt[x_off[0] : x_off[0] + P * sz * TPC].rearrange(
                    "(p f) -> p f", p=P
                )
                engine.dma_start(xf[:].rearrange("p a b -> p (a b)"), src)
                x_off[0] += P * sz * TPC
                for j in range(sz):
                    nc.vector.tensor_scalar(
                        bx[:, k0 + j, :], xf[:, j, :], 0.0, None,
                        op0=mybir.AluOpType.is_gt,
                    )
                    nc.vector.tensor_scalar(
                        bx[:, k0 + j, :], bx[:, k0 + j, :], 2.0, -1.0,
                        op0=mybir.AluOpType.mult, op1=mybir.AluOpType.add,
                    )

            # ---- load + sign phase (issue order == consumption order) ----
            # gpsimd ring: x chunks. sync ring: all W chunks, n0 first
            # (interleaved with x by k-progress), then n1, n2, n3.
            nc.scalar.dma_start(alpha_t[:], al)

            def next_w_ring():
                return nc.sync

            xi = wi = xk = wk = 0
            while xi < len(X_CHUNKS) or wi < len(W_CHUNKS[0]):
                if xi < len(X_CHUNKS) and (wi >= len(W_CHUNKS[0]) or xk <= wk):
                    load_sign_x_chunk(xk, X_CHUNKS[xi], nc.scalar)
                    xk += X_CHUNKS[xi]
                    xi += 1
                else:
                    load_sign_w_chunk(0, wk, W_CHUNKS[0][wi], next_w_ring())
                    wk += W_CHUNKS[0][wi]
                    wi += 1
            for n in (1, 2, 3):
                k0 = 0
                for sz in W_CHUNKS[n]:
                    load_sign_w_chunk(n, k0, sz, next_w_ring())
                    k0 += sz

            def drain(dst, ps, idx, last_pass):
                # DVE drains mid-kernel (ACT is busy signing); alternate
                # DVE/ACT in the last pass so the tail drains in parallel.
                if not last_pass or idx % 2 == 0:
                    nc.vector.tensor_scalar_mul(dst, ps, alpha_t[:])
                else:
                    nc.scalar.activation(
                        dst, ps, mybir.ActivationFunctionType.Copy,
                        scale=alpha_t[:],
                    )

            # ---- matmul phase ----
            for n in range(NT):
                obuf = outp.tile([P, MT, NTS], F32)
                if n < 2:
                    # streaming passes: k-middle / m-inner
                    pss = [
                        ppool.tile([P, NTS], F32, name="ps", tag="ps")
                        for _ in range(MT)
                    ]
                    for k in range(0, KT, K_STEP):
                        for m in range(MT):
                            mm(pss[m][:], m, n, k)
                    for m in range(MT):
                        drain(obuf[:, m, :], pss[m][:], m, n == NT - 1)
                        if m % 2 == 1:
                            nc.scalar.dma_start(
                                out[n, m // 2],
                                obuf[:, m - 1 : m + 1, :].rearrange(
                                    "p a b -> p (a b)"
                                ),
                            )
                else:
                    # resident passes: m-outer / k-inner
                    for m in range(MT):
                        ps = ppool.tile([P, NTS], F32, name="ps", tag="ps")
                        for k in range(0, KT, K_STEP):
                            mm(ps[:], m, n, k)
                        drain(obuf[:, m, :], ps[:], m, n == NT - 1)
                        if m % 2 == 1:
                            nc.scalar.dma_start(
                                out[n, m // 2],
                                obuf[:, m - 1 : m + 1, :].rearrange(
                                    "p a b -> p (a b)"
                                ),
                            )

    nc.compile()
    return nc


def _pack_w(weight):
    # WT4[k, p, n, c] = W^T[(k*128+p), n*512+c]
    wt4 = weight.T.reshape(KT, P, NT, NTS)
    parts = []
    for n in range(NT):
        k0 = 0
        for sz in W_CHUNKS[n]:
            parts.append(
                wt4[k0 : k0 + sz, :, n, :].transpose(1, 0, 2).ravel()
            )
            k0 += sz
    return np.ascontiguousarray(np.concatenate(parts))


def _pack_x_shard(xs):
    # xs: [TPC, INF] -> xT4[k, p, t]
    xt4 = xs.T.reshape(KT, P, TPC)
    parts = []
    k0 = 0
    for sz in X_CHUNKS:
        parts.append(xt4[k0 : k0 + sz].transpose(1, 0, 2).ravel())
        k0 += sz
    return np.ascontiguousarray(np.concatenate(parts))


def kernel(x, weight, alpha):
    global _compiled, LAST_RESULT
    if _compiled is None:
        _compiled = _build()
    nc = _compiled

    x = np.asarray(x, dtype=np.float32)
    weight = np.asarray(weight, dtype=np.float32)
    alpha = np.asarray(alpha, dtype=np.float32)

    wt = _pack_w(weight)
    alv = np.full((P, 1), alpha.reshape(-1)[0], dtype=np.float32)
    in_maps = []
    for c in range(N_CORES):
        xs = _pack_x_shard(x[c * TPC : (c + 1) * TPC, :])
        in_maps.append({"xt": xs, "wt": wt, "alpha": alv})

    LAST_RESULT = run_bass_kernel_spmd(nc, in_maps, list(range(N_CORES)))
    outs = []
    for c in range(N_CORES):
        o = LAST_RESULT.results[c]["out"]  # [NT, MT//2, P, 2*NTS]
        o = o.reshape(NT, MT // 2, P, 2, NTS)
        # -> [MT//2, 2, P, NT, NTS] -> [TPC, OUTF]
        outs.append(o.transpose(1, 3, 2, 0, 4).reshape(TPC, OUTF))
    return np.concatenate(outs, axis=0)
